# revision 8
# baseline (speedup 1.0000x reference)
"""Causal single-head attention (S=8192, dk=64) on 8 TRN2 NeuronCores.

Sharding: zigzag sequence-parallel over query rows. The 8192 rows form 16
blocks of 512; core b owns row-blocks {b, 15-b} so every core does exactly
17 block-sized (512 rows x 512 keys) units of causal work -> perfect load
balance, no collectives.

SPMD constraint (all cores share one instruction graph) is satisfied by
host-side packing: the host packs, per core, 17 "slots" of
(qT, kT, v_aug) operand tiles; slots 0 and 1 are the two diagonal
(triangular-masked) blocks for every core, the remaining 15 are full
blocks. The device graph is identical across cores; only data differs.

Device pipeline per slot (Tile framework handles sync):
  QK^T: 4 matmuls [K=64, M=128 keys, N=512 rows] -> sT in PSUM (f32).
        Slots are processed in two groups mapped to PE row-halves
        (tile_position (0,0) / (64,0)) so pairs of K=64 matmuls can run
        concurrently in the 128-row array.
  exp:  one ACT pass per [128,1024] PSUM tile: bf16 out = exp(s/64),
        fused scale, PSUM -> SBUF.
  mask: slots 0/1 only: gpsimd affine_select zeroes key>row entries.
  AV:   4 matmuls lhsT=v_aug[128 keys, 65] rhs=exp-tile -> PSUM [65,512]
        accumulated over key subtiles; row 64 of v_aug is ones -> row 64
        of the output is the softmax denominator.
  out:  DVE copy PSUM->SBUF, DMA per-slot partial [65,512] to HBM.

Host combines: per row-chunk, sum slot partials, divide by denominator row.
"""

import numpy as np
import ml_dtypes

S = 8192
DK = 64
BLK = 512  # row/key block
NB = S // BLK  # 16
N_CORES = 8
NSLOT = 17  # (b+1) + (16-b) block units per core
G0 = 9  # slots 0..8 -> PE rows 0:64, slots 9..16 -> PE rows 64:128
KSUB = 128  # key subtile (psum partition dim)
NKT = BLK // KSUB  # 4

# process diagonal pairs (0,1) last: their exp->mask->AV chains are longer
PAIR_ORDER = [2, 3, 4, 5, 6, 7, 8, 0, 1]

_BF16 = ml_dtypes.bfloat16
_CACHE = {}

# cubic-in-t fit of exp(t/128) (chebyshev nodes, |t|<=56); the DVE op
# squares it to get exp(t/64). Max rel err ~5.5e-4 for |t|<=56.
_EXPC = (8.02364796e-08, 3.10070749e-05, 7.81220049e-03, 9.99807965e-01)


def _register_exp_dve_op():
    """Register a custom DVE op: out = (((x*c3 + c2)*x + c1)*x + c0)^2.

    One DVE instruction evaluates exp(x/64) to ~5e-4 rel err, letting the
    Vector engine share softmax-exp work with the Scalar engine (the
    per-element-throughput bottleneck of this kernel).
    """
    import numpy as np
    from concourse import dve_ops
    from concourse.dve_spec import (
        Spec, Src0, C0, C1, C2, C3, _spill_c3_to_src1, lower, _has_src1, sq,
    )
    from concourse.dve_uop import DveOpSpec

    name = "EXP_SQ_ANT"
    if name in dve_ops._SUB_OPCODE_FOR_NAME:
        return next(o for o in dve_ops.OPS if o.name == name)

    body = _spill_c3_to_src1(
        sq(((Src0 * C0 + C1) * Src0 + C2) * Src0 + C3))

    def ref(in0, in1, s0, s1, imm2):
        x = in0.astype(np.float32)
        p = ((x * s0 + s1) * x + imm2) * x + in1
        return (p * p).astype(np.float32)

    spec = Spec(body=body, reference=ref)
    row = dve_ops._CUSTOM_DVE_ROW_BASE + len(dve_ops.OPS)
    assert row < 0x20
    shas = {}
    for ver in ("v3",):
        s = DveOpSpec(name=name, opcode=row, uops=lower(spec, ver=ver),
                      rd1_en=_has_src1(spec))
        shas[ver] = s.sha(ver)
    op = dve_ops.DveOp(name, spec, subdim=False, uops_sha=shas)
    dve_ops.OPS.append(op)
    dve_ops._SUB_OPCODE_FOR_NAME[name] = row
    dve_ops.CUSTOM_DVE_SPECS[name] = spec
    return op


def _core_slots(b):
    """Slot table for core b: list of (rowblock, keyblock, is_diag)."""
    A, B = b, 15 - b
    slots = [(A, A, True), (B, B, True)]
    slots += [(A, c, False) for c in range(A)]
    slots += [(B, c, False) for c in range(B)]
    assert len(slots) == NSLOT
    return slots


def _build_graph():
    import concourse.mybir as mybir
    import concourse.tile as tile
    from concourse import bacc

    f32 = mybir.dt.float32
    bf16 = mybir.dt.bfloat16

    exp_op = _register_exp_dve_op()
    d3, d2, d1, d0 = _EXPC

    nc = bacc.Bacc("TRN2", target_bir_lowering=False)
    qp = nc.declare_dram_parameter("qp", [G0, 128, BLK], bf16, isOutput=False)
    kp = nc.declare_dram_parameter("kp", [G0, 128, BLK], bf16, isOutput=False)
    vp = nc.declare_dram_parameter("vp", [NSLOT, 128, NKT * 65], bf16,
                                   isOutput=False)
    op = nc.declare_dram_parameter("op", [NSLOT, 65, BLK], f32, isOutput=True)

    with tile.TileContext(nc) as tc:
        with (
            tc.tile_pool(name="data", bufs=1) as data,
            tc.tile_pool(name="stp", bufs=3, space="PSUM") as stp,
            tc.tile_pool(name="avp", bufs=2, space="PSUM") as avp,
            tc.tile_pool(name="sxp", bufs=8) as sxp,
            tc.tile_pool(name="outp", bufs=3) as outp,
        ):
            # SBUF-resident operands, DMA'd in pair processing order so the
            # first pair's data lands first. q/k on the sync HWDGE ring,
            # v on the scalar ring (two independent FIFOs).
            d0col = data.tile([128, 1], f32, tag="d0col", name="d0col")
            nc.vector.memset(d0col, d0)
            qcol = {}
            kcol = {}
            vcol = {}
            for i in PAIR_ORDER:
                qt = data.tile([128, BLK], bf16, tag=f"q{i}", name=f"qc{i}")
                nc.sync.dma_start(out=qt, in_=qp[i])
                qcol[i] = qt
                kt_ = data.tile([128, BLK], bf16, tag=f"k{i}", name=f"kc{i}")
                nc.sync.dma_start(out=kt_, in_=kp[i])
                kcol[i] = kt_
                for s in (i, 9 + i):
                    if s >= NSLOT:
                        continue
                    vt = data.tile([128, NKT * 65], bf16, tag=f"v{s}",
                                   name=f"vc{s}")
                    nc.scalar.dma_start(out=vt, in_=vp[s])
                    vcol[s] = vt

            for i in PAIR_ORDER:
                slots = [i] + ([9 + i] if 9 + i < NSLOT else [])
                avs = {}
                for half in range(2):
                    sts = {}
                    for s in slots:
                        sts[s] = stp.tile([128, 2 * BLK], f32, tag="st",
                                          name=f"st{s}h{half}")
                    for ktl in range(2):
                        kt = half * 2 + ktl
                        for s in slots:
                            p0 = 0 if s < G0 else 64
                            nc.tensor.matmul(
                                sts[s][:, ktl * BLK:(ktl + 1) * BLK],
                                kcol[i][p0:p0 + 64, kt * KSUB:(kt + 1) * KSUB],
                                qcol[i][p0:p0 + 64, :],
                                start=True,
                                stop=True,
                                tile_position=(p0, 0),
                            )
                    for s in slots:
                        sx = sxp.tile([128, 2 * BLK], bf16, tag="sx",
                                      name=f"sx{s}h{half}")
                        # split exp between DVE (cubic^2 approx) and ACT so
                        # neither engine is the sole per-element bottleneck
                        on_dve = s >= G0 and (half == 1 or i in (2, 3))
                        if on_dve:
                            nc.vector._custom_dve(
                                exp_op, out=sx, in0=sts[s], in1=d0col,
                                s0=d3, s1=d2, imm2=d1,
                            )
                        else:
                            nc.scalar.activation(
                                sx, sts[s], mybir.ActivationFunctionType.Exp,
                                scale=1.0 / DK,
                            )
                        if s < 2:  # diagonal slot: zero keys > row
                            for ktl in range(2):
                                kt = half * 2 + ktl
                                half_ap = sx[:, ktl * BLK:(ktl + 1) * BLK]
                                nc.gpsimd.affine_select(
                                    out=half_ap,
                                    in_=half_ap,
                                    pattern=[[1, BLK]],
                                    compare_op=mybir.AluOpType.is_ge,
                                    fill=0.0,
                                    base=-KSUB * kt,
                                    channel_multiplier=-1,
                                )
                        if half == 0:
                            avs[s] = avp.tile([65, BLK], f32, tag="av",
                                              name=f"av{s}")
                        for ktl in range(2):
                            kt = half * 2 + ktl
                            nc.tensor.matmul(
                                avs[s],
                                vcol[s][:, kt * 65:(kt + 1) * 65],
                                sx[:, ktl * BLK:(ktl + 1) * BLK],
                                start=(kt == 0),
                                stop=(kt == NKT - 1),
                            )
                for s in slots:
                    ot = outp.tile([65, BLK], f32, tag="ot", name=f"ot{s}")
                    nc.vector.tensor_copy(ot, avs[s])
                    nc.sync.dma_start(out=op[s], in_=ot)

    nc.finalize()
    return nc


def _pack_core(q_bf, k_bf, v_bf, b):
    """Build the three packed operand arrays for core b."""
    qp = np.zeros((G0, 128, BLK), dtype=_BF16)
    kp = np.zeros((G0, 128, BLK), dtype=_BF16)
    vp = np.zeros((NSLOT, 128, NKT * 65), dtype=_BF16)
    slots = _core_slots(b)
    for s, (rb, cb, _diag) in enumerate(slots):
        g, i = (0, s) if s < G0 else (1, s - G0)
        p0 = 64 * g
        qp[i, p0:p0 + 64] = q_bf[rb * BLK:(rb + 1) * BLK].T
        kp[i, p0:p0 + 64] = k_bf[cb * BLK:(cb + 1) * BLK].T
        for kt in range(NKT):
            c0 = kt * 65
            vp[s, :, c0:c0 + 64] = (
                v_bf[cb * BLK + kt * KSUB: cb * BLK + (kt + 1) * KSUB])
            vp[s, :, c0 + 64] = np.asarray(1.0, dtype=_BF16)
    return {"qp": qp, "kp": kp, "vp": vp}


def _combine(partials):
    """partials: list of 8 arrays [17, 65, 512] f32 -> full [8192, 64] f32."""
    out = np.empty((S, DK), dtype=np.float32)
    for b in range(N_CORES):
        slots = _core_slots(b)
        for rb in (b, 15 - b):
            idx = [s for s, (r, _c, _d) in enumerate(slots) if r == rb]
            tot = partials[b][idx].sum(axis=0)  # [65, 512]
            out[rb * BLK:(rb + 1) * BLK] = (tot[:DK] / tot[DK]).T
    return out


def kernel(q, k, v):
    from concourse.bass_utils import run_bass_kernel_spmd

    q = np.asarray(q, dtype=np.float32)
    k = np.asarray(k, dtype=np.float32)
    v = np.asarray(v, dtype=np.float32)

    if "nc" not in _CACHE:
        _CACHE["nc"] = _build_graph()
    nc = _CACHE["nc"]

    q_bf = q.astype(_BF16)
    k_bf = k.astype(_BF16)
    v_bf = v.astype(_BF16)
    in_maps = [_pack_core(q_bf, k_bf, v_bf, b) for b in range(N_CORES)]

    res = run_bass_kernel_spmd(nc, in_maps, core_ids=list(range(N_CORES)))
    partials = [np.asarray(res.results[b]["op"], dtype=np.float32)
                for b in range(N_CORES)]
    return _combine(partials)


# revision 11
# speedup vs baseline: 1.4277x; 1.4277x over previous
"""Causal single-head attention (S=8192, dk=64) on 8 TRN2 NeuronCores.

Sharding: zigzag sequence-parallel over query rows. The 8192 rows form 16
blocks of 512; core b owns row-blocks {b, 15-b} so every core does exactly
17 block-sized (512 rows x 512 keys) units of causal work -> perfect load
balance, no collectives.

SPMD constraint (all cores share one instruction graph) is satisfied by
host-side packing: the host packs, per core, 17 "slots" of
(qT, kT, v_aug) operand tiles; slots 0 and 1 are the two diagonal
(triangular-masked) blocks for every core, the remaining 15 are full
blocks. The device graph is identical across cores; only data differs.

Device pipeline per slot (Tile framework handles sync):
  QK^T: 4 matmuls [K=64, M=128 keys, N=512 rows] -> sT in PSUM (f32).
        Slots are processed in two groups mapped to PE row-halves
        (tile_position (0,0) / (64,0)) so pairs of K=64 matmuls can run
        concurrently in the 128-row array.
  exp:  one ACT pass per [128,1024] PSUM tile: bf16 out = exp(s/64),
        fused scale, PSUM -> SBUF.
  mask: slots 0/1 only: gpsimd affine_select zeroes key>row entries.
  AV:   4 matmuls lhsT=v_aug[128 keys, 65] rhs=exp-tile -> PSUM [65,512]
        accumulated over key subtiles; row 64 of v_aug is ones -> row 64
        of the output is the softmax denominator.
  out:  DVE copy PSUM->SBUF, DMA per-slot partial [65,512] to HBM.

Host combines: per row-chunk, sum slot partials, divide by denominator row.
"""

import numpy as np
import ml_dtypes

S = 8192
DK = 64
BLK = 512  # row/key block
NB = S // BLK  # 16
N_CORES = 8
NSLOT = 17  # (b+1) + (16-b) block units per core
G0 = 9  # slots 0..8 -> PE rows 0:64, slots 9..16 -> PE rows 64:128
KSUB = 128  # key subtile (psum partition dim)
NKT = BLK // KSUB  # 4

# diagonal pairs (0,1) have longer exp->mask->AV chains: keep them off the
# cold start and off the drain tail
PAIR_ORDER = [2, 3, 0, 1, 4, 5, 6, 7, 8]

_BF16 = ml_dtypes.bfloat16
_CACHE = {}

# cubic-in-t fit of exp(t/128) (chebyshev nodes, |t|<=56); the DVE op
# squares it to get exp(t/64). Max rel err ~5.5e-4 for |t|<=56.
_EXPC = (8.02364796e-08, 3.10070749e-05, 7.81220049e-03, 9.99807965e-01)


def _register_exp_dve_op():
    """Register a custom DVE op: out = (((x*c3 + c2)*x + c1)*x + c0)^2.

    One DVE instruction evaluates exp(x/64) to ~5e-4 rel err, letting the
    Vector engine share softmax-exp work with the Scalar engine (the
    per-element-throughput bottleneck of this kernel).
    """
    import numpy as np
    from concourse import dve_ops
    from concourse.dve_spec import (
        Spec, Src0, C0, C1, C2, C3, _spill_c3_to_src1, lower, _has_src1, sq,
    )
    from concourse.dve_uop import DveOpSpec

    name = "EXP_SQ_ANT"
    if name in dve_ops._SUB_OPCODE_FOR_NAME:
        return next(o for o in dve_ops.OPS if o.name == name)

    body = _spill_c3_to_src1(
        sq(((Src0 * C0 + C1) * Src0 + C2) * Src0 + C3))

    def ref(in0, in1, s0, s1, imm2):
        x = in0.astype(np.float32)
        p = ((x * s0 + s1) * x + imm2) * x + in1
        return (p * p).astype(np.float32)

    spec = Spec(body=body, reference=ref)
    row = dve_ops._CUSTOM_DVE_ROW_BASE + len(dve_ops.OPS)
    assert row < 0x20
    shas = {}
    for ver in ("v3",):
        s = DveOpSpec(name=name, opcode=row, uops=lower(spec, ver=ver),
                      rd1_en=_has_src1(spec))
        shas[ver] = s.sha(ver)
    op = dve_ops.DveOp(name, spec, subdim=False, uops_sha=shas)
    dve_ops.OPS.append(op)
    dve_ops._SUB_OPCODE_FOR_NAME[name] = row
    dve_ops.CUSTOM_DVE_SPECS[name] = spec
    return op


def _core_slots(b):
    """Slot table for core b: list of (rowblock, keyblock, is_diag)."""
    A, B = b, 15 - b
    slots = [(A, A, True), (B, B, True)]
    slots += [(A, c, False) for c in range(A)]
    slots += [(B, c, False) for c in range(B)]
    assert len(slots) == NSLOT
    return slots


def _build_graph():
    import concourse.mybir as mybir
    import concourse.tile as tile
    from concourse import bacc

    f32 = mybir.dt.float32
    bf16 = mybir.dt.bfloat16

    exp_op = _register_exp_dve_op()
    d3, d2, d1, d0 = _EXPC

    nc = bacc.Bacc("TRN2", target_bir_lowering=False)
    qp = nc.declare_dram_parameter("qp", [G0, 128, BLK], bf16, isOutput=False)
    kp = nc.declare_dram_parameter("kp", [G0, 128, BLK], bf16, isOutput=False)
    vp = nc.declare_dram_parameter("vp", [NSLOT, 128, NKT * 65], bf16,
                                   isOutput=False)
    op = nc.declare_dram_parameter("op", [NSLOT, 65, BLK], f32, isOutput=True)

    with tile.TileContext(nc) as tc:
        with (
            tc.tile_pool(name="data", bufs=1) as data,
            tc.tile_pool(name="stp", bufs=3, space="PSUM") as stp,
            tc.tile_pool(name="avp", bufs=2, space="PSUM") as avp,
            tc.tile_pool(name="sxp", bufs=8) as sxp,
            tc.tile_pool(name="outp", bufs=3) as outp,
        ):
            # SBUF-resident operands, DMA'd in pair processing order so the
            # first pair's data lands first. q/k on the sync HWDGE ring,
            # v on the scalar ring (two independent FIFOs).
            d0col = data.tile([128, 1], f32, tag="d0col", name="d0col")
            nc.vector.memset(d0col, d0)
            qcol = {}
            kcol = {}
            vcol = {}
            for i in PAIR_ORDER:
                qt = data.tile([128, BLK], bf16, tag=f"q{i}", name=f"qc{i}")
                nc.sync.dma_start(out=qt, in_=qp[i])
                qcol[i] = qt
                kt_ = data.tile([128, BLK], bf16, tag=f"k{i}", name=f"kc{i}")
                nc.sync.dma_start(out=kt_, in_=kp[i])
                kcol[i] = kt_
                for s in (i, 9 + i):
                    if s >= NSLOT:
                        continue
                    vt = data.tile([128, NKT * 65], bf16, tag=f"v{s}",
                                   name=f"vc{s}")
                    nc.sync.dma_start(out=vt, in_=vp[s])
                    vcol[s] = vt

            for i in PAIR_ORDER:
                slots = [i] + ([9 + i] if 9 + i < NSLOT else [])
                sxs = {s: [] for s in slots}
                for half in range(2):
                    sts = {}
                    for s in slots:
                        sts[s] = stp.tile([128, 2 * BLK], f32, tag="st",
                                          name=f"st{s}h{half}")
                    for ktl in range(2):
                        kt = half * 2 + ktl
                        for s in slots:
                            p0 = 0 if s < G0 else 64
                            nc.tensor.matmul(
                                sts[s][:, ktl * BLK:(ktl + 1) * BLK],
                                kcol[i][p0:p0 + 64, kt * KSUB:(kt + 1) * KSUB],
                                qcol[i][p0:p0 + 64, :],
                                start=True,
                                stop=True,
                                tile_position=(p0, 0),
                            )
                    for s in slots:
                        sx = sxp.tile([128, 2 * BLK], bf16, tag="sx",
                                      name=f"sx{s}h{half}")
                        # split exp between DVE (cubic^2 approx) and ACT so
                        # neither engine is the sole per-element bottleneck
                        on_dve = s >= G0 and (half == 1 or i in (2, 3))
                        if on_dve:
                            nc.vector._custom_dve(
                                exp_op, out=sx, in0=sts[s], in1=d0col,
                                s0=d3, s1=d2, imm2=d1,
                            )
                        else:
                            nc.scalar.activation(
                                sx, sts[s], mybir.ActivationFunctionType.Exp,
                                scale=1.0 / DK,
                            )
                        if s < 2:  # diagonal slot: zero keys > row
                            for ktl in range(2):
                                kt = half * 2 + ktl
                                half_ap = sx[:, ktl * BLK:(ktl + 1) * BLK]
                                nc.gpsimd.affine_select(
                                    out=half_ap,
                                    in_=half_ap,
                                    pattern=[[1, BLK]],
                                    compare_op=mybir.AluOpType.is_ge,
                                    fill=0.0,
                                    base=-KSUB * kt,
                                    channel_multiplier=-1,
                                )
                        sxs[s].append(sx)
                for s in slots:
                    av = avp.tile([65, BLK], f32, tag="av", name=f"av{s}")
                    for kt in range(NKT):
                        nc.tensor.matmul(
                            av,
                            vcol[s][:, kt * 65:(kt + 1) * 65],
                            sxs[s][kt // 2][:, (kt % 2) * BLK:(kt % 2 + 1) * BLK],
                            start=(kt == 0),
                            stop=(kt == NKT - 1),
                        )
                    ot = outp.tile([65, BLK], f32, tag="ot", name=f"ot{s}")
                    nc.vector.tensor_copy(ot, av)
                    nc.sync.dma_start(out=op[s], in_=ot)

    nc.finalize()
    return nc


def _pack_core(q_bf, k_bf, v_bf, b):
    """Build the three packed operand arrays for core b."""
    qp = np.zeros((G0, 128, BLK), dtype=_BF16)
    kp = np.zeros((G0, 128, BLK), dtype=_BF16)
    vp = np.zeros((NSLOT, 128, NKT * 65), dtype=_BF16)
    slots = _core_slots(b)
    for s, (rb, cb, _diag) in enumerate(slots):
        g, i = (0, s) if s < G0 else (1, s - G0)
        p0 = 64 * g
        qp[i, p0:p0 + 64] = q_bf[rb * BLK:(rb + 1) * BLK].T
        kp[i, p0:p0 + 64] = k_bf[cb * BLK:(cb + 1) * BLK].T
        for kt in range(NKT):
            c0 = kt * 65
            vp[s, :, c0:c0 + 64] = (
                v_bf[cb * BLK + kt * KSUB: cb * BLK + (kt + 1) * KSUB])
            vp[s, :, c0 + 64] = np.asarray(1.0, dtype=_BF16)
    return {"qp": qp, "kp": kp, "vp": vp}


def _combine(partials):
    """partials: list of 8 arrays [17, 65, 512] f32 -> full [8192, 64] f32."""
    out = np.empty((S, DK), dtype=np.float32)
    for b in range(N_CORES):
        slots = _core_slots(b)
        for rb in (b, 15 - b):
            idx = [s for s, (r, _c, _d) in enumerate(slots) if r == rb]
            tot = partials[b][idx].sum(axis=0)  # [65, 512]
            out[rb * BLK:(rb + 1) * BLK] = (tot[:DK] / tot[DK]).T
    return out


def kernel(q, k, v):
    from concourse.bass_utils import run_bass_kernel_spmd

    q = np.asarray(q, dtype=np.float32)
    k = np.asarray(k, dtype=np.float32)
    v = np.asarray(v, dtype=np.float32)

    if "nc" not in _CACHE:
        _CACHE["nc"] = _build_graph()
    nc = _CACHE["nc"]

    q_bf = q.astype(_BF16)
    k_bf = k.astype(_BF16)
    v_bf = v.astype(_BF16)
    in_maps = [_pack_core(q_bf, k_bf, v_bf, b) for b in range(N_CORES)]

    res = run_bass_kernel_spmd(nc, in_maps, core_ids=list(range(N_CORES)))
    partials = [np.asarray(res.results[b]["op"], dtype=np.float32)
                for b in range(N_CORES)]
    return _combine(partials)


# revision 13
# speedup vs baseline: 1.8843x; 1.3198x over previous
"""Causal single-head attention (S=8192, dk=64) on 8 TRN2 NeuronCores.

Sharding: zigzag sequence-parallel over query rows. The 8192 rows form 16
blocks of 512; core b owns row-blocks {b, 15-b} so every core does exactly
17 block-sized (512 rows x 512 keys) units of causal work -> perfect load
balance, no collectives.

SPMD constraint (all cores share one instruction graph) is satisfied by
host-side packing: the host packs, per core, 17 "slots" of
(qT, kT, v_aug) operand tiles; slots 0 and 1 are the two diagonal
(triangular-masked) blocks for every core, the remaining 15 are full
blocks. The device graph is identical across cores; only data differs.
Slots are processed as 9 pairs (slot i, slot 9+i) mapped to the two
PE-array row-halves so the K=64 QK^T matmuls run two-at-a-time.

Device pipeline per pair (Tile framework handles sync):
  QK^T: per key-subtile: two concurrent matmuls [K=64, M=128, N=512]
        (tile_position (0,0)/(64,0)) -> sT [128,512] f32 in PSUM.
  exp:  exp(s/64) -> bf16 SBUF, split between ACT (exact, fused scale)
        and a custom DVE op ((cubic)^2 approx) so both engines share the
        per-element softmax work.
  mask: slots 0/1 only: gpsimd affine_select zeroes key>row entries.
  AV:   per slot: 4 matmuls lhsT=v_aug[128 keys, 65] rhs=exp tile,
        accumulated in PSUM [65, 1024] (both slots of the pair share the
        tile, one 512-col half each); row 64 of v_aug is ones so row 64
        is the softmax denominator.
  out:  one DVE copy [65,1024] PSUM->SBUF + one DMA per pair.

Host combines: per row-chunk, sum slot partials, divide by denominator row.
"""

import numpy as np
import ml_dtypes

S = 8192
DK = 64
BLK = 512  # row/key block
NB = S // BLK  # 16
N_CORES = 8
NSLOT = 17  # (b+1) + (16-b) block units per core
G0 = 9  # slots 0..8 -> PE rows 0:64, slots 9..16 -> PE rows 64:128
NPAIR = 9
KSUB = 128  # key subtile (psum partition dim)
NKT = BLK // KSUB  # 4
VW = NKT * 65  # 260

# diagonal pairs (0,1) have longer exp->mask->AV chains: keep them off the
# cold start and off the drain tail
PAIR_ORDER = [2, 3, 0, 1, 4, 5, 6, 7, 8]

_BF16 = ml_dtypes.bfloat16
_CACHE = {}

# cubic-in-t fit of exp(t/128) (chebyshev nodes, |t|<=56); the DVE op
# squares it to get exp(t/64). Max rel err ~5.5e-4 for |t|<=56.
_EXPC = (8.02364796e-08, 3.10070749e-05, 7.81220049e-03, 9.99807965e-01)


def _register_exp_dve_op():
    """Register a custom DVE op: out = (((x*c3 + c2)*x + c1)*x + c0)^2.

    One DVE instruction evaluates exp(x/64) to ~5e-4 rel err, letting the
    Vector engine share softmax-exp work with the Scalar engine (the
    per-element-throughput bottleneck of this kernel).
    """
    import numpy as np
    from concourse import dve_ops
    from concourse.dve_spec import (
        Spec, Src0, C0, C1, C2, C3, _spill_c3_to_src1, lower, _has_src1, sq,
    )
    from concourse.dve_uop import DveOpSpec

    name = "EXP_SQ_ANT"
    if name in dve_ops._SUB_OPCODE_FOR_NAME:
        return next(o for o in dve_ops.OPS if o.name == name)

    body = _spill_c3_to_src1(
        sq(((Src0 * C0 + C1) * Src0 + C2) * Src0 + C3))

    def ref(in0, in1, s0, s1, imm2):
        x = in0.astype(np.float32)
        p = ((x * s0 + s1) * x + imm2) * x + in1
        return (p * p).astype(np.float32)

    spec = Spec(body=body, reference=ref)
    row = dve_ops._CUSTOM_DVE_ROW_BASE + len(dve_ops.OPS)
    assert row < 0x20
    shas = {}
    for ver in ("v3",):
        s = DveOpSpec(name=name, opcode=row, uops=lower(spec, ver=ver),
                      rd1_en=_has_src1(spec))
        shas[ver] = s.sha(ver)
    op = dve_ops.DveOp(name, spec, subdim=False, uops_sha=shas)
    dve_ops.OPS.append(op)
    dve_ops._SUB_OPCODE_FOR_NAME[name] = row
    dve_ops.CUSTOM_DVE_SPECS[name] = spec
    return op


def _core_slots(b):
    """Slot table for core b: list of (rowblock, keyblock, is_diag)."""
    A, B = b, 15 - b
    slots = [(A, A, True), (B, B, True)]
    slots += [(A, c, False) for c in range(A)]
    slots += [(B, c, False) for c in range(B)]
    assert len(slots) == NSLOT
    return slots


def _on_dve(s, kt):
    """Which exp tiles run on the Vector engine (cubic^2) vs Scalar (exact).

    Chosen to balance ACT total (43 tiles + table load) against DVE total
    (25 tiles + 9 output copies). Early-kt tiles go to DVE so its work is
    off the pair-tail critical path.
    """
    return kt == 0 or (kt == 1 and s >= G0)


def _build_graph():
    import concourse.mybir as mybir
    import concourse.tile as tile
    from concourse import bacc

    f32 = mybir.dt.float32
    bf16 = mybir.dt.bfloat16

    exp_op = _register_exp_dve_op()
    d3, d2, d1, d0 = _EXPC

    nc = bacc.Bacc("TRN2", target_bir_lowering=False)
    # qk: per pair, q^T strip then k^T strip (each [128, 512], two slots
    # stacked on the partition axis)
    qkp = nc.declare_dram_parameter("qkp", [NPAIR, 128, 2 * BLK], bf16,
                                    isOutput=False)
    vp = nc.declare_dram_parameter("vp", [NPAIR, 128, 2 * VW], bf16,
                                   isOutput=False)
    op = nc.declare_dram_parameter("op", [NPAIR, 65, 2 * BLK], f32,
                                   isOutput=True)

    with tile.TileContext(nc) as tc:
        with (
            tc.tile_pool(name="data", bufs=1) as data,
            tc.tile_pool(name="stp", bufs=6, space="PSUM") as stp,
            tc.tile_pool(name="avp", bufs=1, space="PSUM") as avp,
            tc.tile_pool(name="sxp", bufs=12) as sxp,
            tc.tile_pool(name="outp", bufs=2) as outp,
        ):
            d0col = data.tile([128, 1], f32, tag="d0col", name="d0col")
            nc.vector.memset(d0col, d0)
            qkcol = {}
            vcol = {}
            for i in PAIR_ORDER:
                t = data.tile([128, 2 * BLK], bf16, tag=f"qk{i}",
                              name=f"qk{i}")
                nc.sync.dma_start(out=t, in_=qkp[i])
                qkcol[i] = t
                vt = data.tile([128, 2 * VW], bf16, tag=f"v{i}",
                               name=f"vc{i}")
                nc.sync.dma_start(out=vt, in_=vp[i])
                vcol[i] = vt

            for i in PAIR_ORDER:
                slots = [i] + ([9 + i] if 9 + i < NSLOT else [])
                sxs = {s: [] for s in slots}
                for kt in range(NKT):
                    sts = {}
                    for s in slots:
                        p0 = 0 if s < G0 else 64
                        st = stp.tile([128, BLK], f32, tag="st",
                                      name=f"st{s}k{kt}")
                        nc.tensor.matmul(
                            st,
                            qkcol[i][p0:p0 + 64,
                                     BLK + kt * KSUB:BLK + (kt + 1) * KSUB],
                            qkcol[i][p0:p0 + 64, 0:BLK],
                            start=True,
                            stop=True,
                            tile_position=(p0, 0),
                        )
                        sts[s] = st
                    for s in slots:
                        sx = sxp.tile([128, BLK], bf16, tag="sx",
                                      name=f"sx{s}k{kt}")
                        if _on_dve(s, kt):
                            nc.vector._custom_dve(
                                exp_op, out=sx, in0=sts[s], in1=d0col,
                                s0=d3, s1=d2, imm2=d1,
                            )
                        else:
                            nc.scalar.activation(
                                sx, sts[s], mybir.ActivationFunctionType.Exp,
                                scale=1.0 / DK,
                            )
                        if s < 2:  # diagonal slot: zero keys > row
                            nc.gpsimd.affine_select(
                                out=sx,
                                in_=sx,
                                pattern=[[1, BLK]],
                                compare_op=mybir.AluOpType.is_ge,
                                fill=0.0,
                                base=-KSUB * kt,
                                channel_multiplier=-1,
                            )
                        sxs[s].append(sx)
                av = avp.tile([65, 2 * BLK], f32, tag="av", name=f"av{i}")
                for s in slots:
                    off = 0 if s == i else BLK
                    for kt in range(NKT):
                        nc.tensor.matmul(
                            av[:, off:off + BLK],
                            vcol[i][:, (0 if s == i else VW) + kt * 65:
                                    (0 if s == i else VW) + (kt + 1) * 65],
                            sxs[s][kt],
                            start=(kt == 0),
                            stop=(kt == NKT - 1),
                        )
                w = 2 * BLK if len(slots) == 2 else BLK
                ot = outp.tile([65, 2 * BLK], f32, tag="ot", name=f"ot{i}")
                nc.vector.tensor_copy(ot[:, 0:w], av[:, 0:w])
                nc.sync.dma_start(out=op[i][:, 0:w], in_=ot[:, 0:w])

    nc.finalize()
    return nc


def _pack_core(q_bf, k_bf, v_bf, b):
    """Build the packed operand arrays for core b."""
    qkp = np.zeros((NPAIR, 128, 2 * BLK), dtype=_BF16)
    vp = np.zeros((NPAIR, 128, 2 * VW), dtype=_BF16)
    slots = _core_slots(b)
    for s, (rb, cb, _diag) in enumerate(slots):
        i = s if s < G0 else s - G0
        p0 = 0 if s < G0 else 64
        voff = 0 if s < G0 else VW
        qkp[i, p0:p0 + 64, 0:BLK] = q_bf[rb * BLK:(rb + 1) * BLK].T
        qkp[i, p0:p0 + 64, BLK:2 * BLK] = k_bf[cb * BLK:(cb + 1) * BLK].T
        for kt in range(NKT):
            c0 = voff + kt * 65
            vp[i, :, c0:c0 + 64] = (
                v_bf[cb * BLK + kt * KSUB: cb * BLK + (kt + 1) * KSUB])
            vp[i, :, c0 + 64] = np.asarray(1.0, dtype=_BF16)
    return {"qkp": qkp, "vp": vp}


def _slot_partial(op_arr, s):
    """Extract slot s's [65, 512] partial from the per-pair output array."""
    i = s if s < G0 else s - G0
    off = 0 if s < G0 else BLK
    return op_arr[i, :, off:off + BLK]


def _combine(partials):
    """partials: list of 8 arrays [9, 65, 1024] f32 -> full [8192, 64] f32."""
    out = np.empty((S, DK), dtype=np.float32)
    for b in range(N_CORES):
        slots = _core_slots(b)
        for rb in (b, 15 - b):
            idx = [s for s, (r, _c, _d) in enumerate(slots) if r == rb]
            tot = np.zeros((65, BLK), dtype=np.float32)
            for s in idx:
                tot += _slot_partial(partials[b], s)
            out[rb * BLK:(rb + 1) * BLK] = (tot[:DK] / tot[DK]).T
    return out


def kernel(q, k, v):
    from concourse.bass_utils import run_bass_kernel_spmd

    q = np.asarray(q, dtype=np.float32)
    k = np.asarray(k, dtype=np.float32)
    v = np.asarray(v, dtype=np.float32)

    if "nc" not in _CACHE:
        _CACHE["nc"] = _build_graph()
    nc = _CACHE["nc"]

    q_bf = q.astype(_BF16)
    k_bf = k.astype(_BF16)
    v_bf = v.astype(_BF16)
    in_maps = [_pack_core(q_bf, k_bf, v_bf, b) for b in range(N_CORES)]

    res = run_bass_kernel_spmd(nc, in_maps, core_ids=list(range(N_CORES)))
    partials = [np.asarray(res.results[b]["op"], dtype=np.float32)
                for b in range(N_CORES)]
    return _combine(partials)
